# revision 1
# baseline (speedup 1.0000x reference)
"""CrossAttention Trainium2 kernel — 8-core SPMD, no collectives.

Sharding: core c = (p, s) with p = c // 2 (head pair {2p, 2p+1}),
s = c % 2 (query shard: image rows [32s, 32s+32), tokens [2048s, +2048)).

Host composes the 3x3 convs with the q/k/v projections (both linear):
  k = conv(akv, Wk@kvw), v = conv(akv, Wv@kvw), q = SCALE*conv(x, Wq@qw)
so each core runs ONE 128-out-ch conv over full akv
([k_h0 pad32 | k_h1 pad32 | v_h0 | v_h1]) and one 64-out-ch conv over its
34-row x slice. Attention is flash-style: per 128-key block, two row-tiled
QK^T matmuls into a double-buffered 2-bank PSUM group (strips alternate by
key-block parity for 4-way PE tiling), one ACT exp over [128, 1024], AV
with a fused ones-column denominator. The scrambled output reshape
(o[h,n,d] -> rows n2=512h+n//8, ch (n%8)*32+d) is realized with
shift-matmuls against host-built shifted identities.

Per-chunk tiles keep dependencies fine-grained so attention overlaps the
conv. ACT runs exp only; all copies are on DVE. Matmul dtypes: fp32r for
convs (DMA-fed), bf16 for the attention path.
"""

import numpy as np

import concourse.bass as bass
import concourse.mybir as mybir
import concourse.tile as tile
from concourse import bacc, bass_utils

DIM = 256
HEADS = 8
HEAD_DIM = 32          # v head dim
QK_DIM = 16            # q/k head dim
SCALE = HEAD_DIM ** -0.5
H = W = 64
N = H * W              # 4096 tokens
N_CORES = 8
NQ = N // 2            # queries per core (2048)
KB = 128               # key block size
N_KB = N // KB         # 32 key blocks
QC = 512               # query chunk (matmul N)
N_QC = NQ // QC        # 4 query chunks

F32 = mybir.dt.float32
F32R = mybir.dt.float32r
BF16 = mybir.dt.bfloat16
AF = mybir.ActivationFunctionType

_CACHE = {}


def build_nc(n_reps: int = 1):
    """Build + compile the SPMD Bass program (same NEFF on all 8 cores)."""
    key = ("nc", n_reps)
    if key in _CACHE:
        return _CACHE[key]
    nc = bacc.Bacc("TRN2", target_bir_lowering=False, debug=False,
                   num_devices=N_CORES)

    d = {}
    d["akv"] = nc.dram_tensor("akv", [DIM, 66 * 66], F32R, kind="ExternalInput").ap()
    d["xs"] = nc.dram_tensor("xs", [DIM, 34 * 66], F32R, kind="ExternalInput").ap()
    d["wkv"] = nc.dram_tensor("wkv", [DIM, 9, 128], F32R, kind="ExternalInput").ap()
    d["wq"] = nc.dram_tensor("wq", [DIM, 9, 64], F32R, kind="ExternalInput").ap()
    d["bkv"] = nc.dram_tensor("bkv", [1, 128], F32R, kind="ExternalInput").ap()
    d["bq"] = nc.dram_tensor("bq", [1, 64], F32R, kind="ExternalInput").ap()
    d["ones512"] = nc.dram_tensor("ones512", [1, 512], F32R, kind="ExternalInput").ap()
    d["ones32"] = nc.dram_tensor("ones32", [1, 32], F32R, kind="ExternalInput").ap()
    d["ident64"] = nc.dram_tensor("ident64", [64, 64], F32, kind="ExternalInput").ap()
    d["identsh"] = nc.dram_tensor("identsh", [32, 4, 128], BF16, kind="ExternalInput").ap()
    d["wpt"] = nc.dram_tensor("wpt", [DIM, DIM], BF16, kind="ExternalInput").ap()
    d["bp"] = nc.dram_tensor("bp", [128, 2], F32, kind="ExternalInput").ap()
    out_d = nc.dram_tensor("out", [DIM, 512], F32, kind="ExternalOutput").ap()

    with tile.TileContext(nc) as tc:
        if n_reps == 1:
            _emit(nc, tc, d, out_d)
        else:
            with tc.For_i(0, n_reps, 1):
                _emit(nc, tc, d, out_d)

    nc.compile()
    _CACHE[key] = nc
    return nc


def _emit(nc, tc, d, out_d):
    import contextlib
    ctx = contextlib.ExitStack()
    with ctx:
        consts = ctx.enter_context(tc.tile_pool(name="consts", bufs=1))
        big = ctx.enter_context(tc.tile_pool(name="big", bufs=1))
        epool = ctx.enter_context(tc.tile_pool(name="epool", bufs=3))
        small = ctx.enter_context(tc.tile_pool(name="small", bufs=4))
        qk_ps = ctx.enter_context(tc.tile_pool(name="qk_ps", bufs=2, space="PSUM"))
        av_ps = ctx.enter_context(tc.tile_pool(name="av_ps", bufs=2, space="PSUM"))
        aux_ps = ctx.enter_context(tc.tile_pool(name="aux_ps", bufs=2, space="PSUM"))

        # ---- constants / weights -------------------------------------------
        wkv_sb = consts.tile([128, 2, 9, 128], F32R, tag="wkv_sb")
        nc.sync.dma_start(out=wkv_sb,
                          in_=d["wkv"].rearrange("(b p) t o -> p b t o", p=128))
        wq_sb = consts.tile([128, 2, 9, 64], F32R, tag="wq_sb")
        nc.sync.dma_start(out=wq_sb,
                          in_=d["wq"].rearrange("(b p) t o -> p b t o", p=128))
        bkv_sb = consts.tile([1, 128], F32R, tag="bkv_sb")
        nc.sync.dma_start(out=bkv_sb, in_=d["bkv"])
        bq_sb = consts.tile([1, 64], F32R, tag="bq_sb")
        nc.sync.dma_start(out=bq_sb, in_=d["bq"])
        ones512 = consts.tile([1, 512], F32R, tag="ones512")
        nc.sync.dma_start(out=ones512, in_=d["ones512"])
        ones32 = consts.tile([1, 32], F32R, tag="ones32")
        nc.sync.dma_start(out=ones32, in_=d["ones32"])
        ident64 = consts.tile([128, 64], F32, tag="ident64")
        nc.sync.dma_start(out=ident64[64:128, :], in_=d["ident64"])
        identsh = consts.tile([32, 4, 128], BF16, tag="identsh")
        nc.sync.dma_start(out=identsh, in_=d["identsh"])
        wpt_sb = consts.tile([128, 2, 256], BF16, tag="wpt_sb")
        nc.sync.dma_start(out=wpt_sb,
                          in_=d["wpt"].rearrange("(b p) o -> p b o", p=128))
        bp_sb = consts.tile([128, 2], F32, tag="bp_sb")
        nc.sync.dma_start(out=bp_sb, in_=d["bp"])

        # ---- inputs ---------------------------------------------------------
        akv_sb = big.tile([128, 2, 66, 66], F32R, tag="akv_sb")
        nc.sync.dma_start(out=akv_sb,
                          in_=d["akv"].rearrange("(b p) (y x) -> p b y x", p=128, y=66))
        xs_sb = big.tile([128, 2, 34, 66], F32R, tag="xs_sb")
        nc.sync.dma_start(out=xs_sb,
                          in_=d["xs"].rearrange("(b p) (y x) -> p b y x", p=128, y=34))

        # ---- persistent intermediates (per-chunk tiles for fine deps) -------
        kT = [big.tile([128, 512], BF16, tag=f"kT{c}", name=f"kT{c}")
              for c in range(8)]
        qTt = [big.tile([128, 512], BF16, tag=f"qT{c}", name=f"qT{c}")
               for c in range(4)]
        vT = [big.tile([128, 512], F32, tag=f"vT{c}", name=f"vT{c}")
              for c in range(8)]
        Vt = [big.tile([128, 4, 66], BF16, tag=f"V{c}", name=f"V{c}")
              for c in range(8)]
        o_nrm = big.tile([32, 2, NQ], BF16, tag="o_nrm")
        rowsT = big.tile([128, 2, 512], BF16, tag="rowsT")

        for c in range(8):
            nc.vector.memset(Vt[c][:, :, 32:33], 1.0)
            nc.vector.memset(Vt[c][:, :, 65:66], 1.0)

        # ---- kv conv: akv -> [k_h0pad32 | k_h1pad32 | v_h0 | v_h1] ----------
        for c in range(8):                                  # 8 pixel chunks of 512
            ps = aux_ps.tile([128, 512], F32, tag="aux", name=f"cvkv{c}")
            first = True
            for cib in range(2):
                for ky in range(3):
                    for kx in range(3):
                        rhs = akv_sb[:, cib, c * 8 + ky: c * 8 + ky + 8, kx: kx + 64]
                        nc.tensor.matmul(
                            ps, wkv_sb[:, cib, ky * 3 + kx, :], rhs,
                            start=first, stop=False, skip_group_check=True)
                        first = False
            nc.tensor.matmul(ps, bkv_sb, ones512, start=False, stop=True,
                             skip_group_check=True)
            # rows 0:64 -> kT strips (bf16); rows 64:128 -> vT (f32)
            nc.vector.tensor_copy(kT[c][0:64, :], ps[0:64, :])
            nc.sync.dma_start(out=kT[c][64:128, :], in_=kT[c][0:64, :])
            nc.vector.tensor_copy(vT[c][64:128, :], ps[64:128, :])
            # transpose v for the 4 key blocks of this chunk
            for j in range(4):
                tp = aux_ps.tile([128, 512], F32, tag="aux", name=f"tp{c}_{j}")
                nc.tensor.transpose(tp[:, 0:64],
                                    vT[c][64:128, j * 128:(j + 1) * 128],
                                    ident64[64:128, :])
                nc.vector.tensor_copy(out=Vt[c][:, j, 0:32], in_=tp[:, 0:32])
                nc.vector.tensor_copy(out=Vt[c][:, j, 33:65], in_=tp[:, 32:64])

        # ---- q conv: xs -> [q_h0 pad32 | q_h1 pad32] ------------------------
        for c in range(4):                                  # 4 chunks of 512
            ps = aux_ps.tile([128, 512], F32, tag="aux", name=f"cvq{c}")
            first = True
            for cib in range(2):
                for ky in range(3):
                    for kx in range(3):
                        rhs = xs_sb[:, cib, c * 8 + ky: c * 8 + ky + 8, kx: kx + 64]
                        nc.tensor.matmul(
                            ps[0:64, :], wq_sb[:, cib, ky * 3 + kx, :], rhs,
                            start=first, stop=False, skip_group_check=True)
                        first = False
            nc.tensor.matmul(ps[0:64, :], bq_sb, ones512, start=False, stop=True,
                             skip_group_check=True)
            nc.vector.tensor_copy(qTt[c][0:64, :], ps[0:64, :])
            nc.sync.dma_start(out=qTt[c][64:128, :], in_=qTt[c][0:64, :])

        # ---- attention ------------------------------------------------------
        for qc in range(N_QC):
            acc = [av_ps.tile([33, 512], F32, tag="av", name=f"acc{qc}_{i}")
                   for i in range(2)]
            for kb in range(N_KB):
                sp = 2 * (kb % 2)                           # strip pair 0/2
                cc, j = kb // 4, kb % 4
                lg = qk_ps.tile([128, 2, 512], F32, tag="qk", name=f"lg{qc}_{kb}")
                for hl in range(2):
                    i = sp + hl
                    nc.tensor.matmul(
                        lg[:, hl, :],
                        kT[cc][32 * i:32 * i + 32, j * 128:(j + 1) * 128],
                        qTt[qc][32 * i:32 * i + 32, :],
                        start=True, stop=True, skip_group_check=True,
                        tile_position=(32 * i, 0))
                eg = epool.tile([128, 2, 512], BF16, tag="eg", name=f"eg{qc}_{kb}")
                nc.scalar.activation(eg, lg, AF.Exp)
                for hl in range(2):
                    nc.tensor.matmul(
                        acc[hl], Vt[cc][:, j, 33 * hl: 33 * hl + 33],
                        eg[:, hl, :],
                        start=(kb == 0), stop=(kb == N_KB - 1),
                        skip_group_check=True)
            qsl = slice(qc * 512, (qc + 1) * 512)
            for hl in range(2):
                r = small.tile([1, 512], F32R, tag="recip", name=f"r{qc}_{hl}")
                with nc.allow_low_precision(reason="fp32r recip is ~19-bit"):
                    nc.vector.reciprocal(r, acc[hl][32:33, :])
                rb = aux_ps.tile([128, 512], F32, tag="aux", name=f"rb{qc}_{hl}")
                nc.tensor.matmul(rb[0:32, :], ones32, r, start=True, stop=True,
                                 skip_group_check=True)
                rbs = small.tile([32, 512], F32, tag="rbs", name=f"rbs{qc}_{hl}")
                nc.vector.tensor_copy(rbs, rb[0:32, :])
                nc.vector.tensor_mul(o_nrm[:, hl, qsl],
                                     acc[hl][0:32, :], rbs)

        # ---- scramble shuffle (shift-matmuls) + output projection -----------
        o_j = o_nrm.rearrange("p h (m j) -> p h j m", j=8)   # [32, 2, 8, 256]
        for icb in range(2):
            rp = aux_ps.tile([128, 512], F32, tag="aux", name=f"rp{icb}")
            for hl in range(2):
                for jm in range(4):
                    j = 4 * icb + jm
                    nc.tensor.matmul(
                        rp[:, 256 * hl:256 * hl + 256],
                        identsh[:, jm, :], o_j[:, hl, j, :],
                        start=(jm == 0), stop=(jm == 3), skip_group_check=True)
            nc.vector.tensor_copy(rowsT[:, icb, :], rp)
        for ocb in range(2):
            po = aux_ps.tile([128, 512], F32, tag="aux", name=f"po{ocb}")
            for icb in range(2):
                nc.tensor.matmul(po, wpt_sb[:, icb, 128 * ocb:128 * ocb + 128],
                                 rowsT[:, icb, :], start=(icb == 0),
                                 stop=(icb == 1), skip_group_check=True)
            ob = small.tile([128, 512], F32, tag="outsb", name=f"ob{ocb}")
            nc.vector.tensor_scalar_add(ob, po, bp_sb[:, ocb:ocb + 1])
            nc.sync.dma_start(out=out_d[128 * ocb:128 * ocb + 128, :], in_=ob)


# --------------------------------------------------------------------------
# host side
# --------------------------------------------------------------------------

def host_prep(x, attn_kv, qw, qb, kvw, kvb, Wq, bq, Wk, bk, Wv, bv, Wp, bp):
    import ml_dtypes
    f = np.float32
    x = np.asarray(x, f)[0]          # [256, 64, 64]
    akv = np.asarray(attn_kv, f)[0]
    Wqc = np.einsum("jc,ciyx->jiyx", np.asarray(Wq, f), np.asarray(qw, f)) * SCALE
    Wkc = np.einsum("jc,ciyx->jiyx", np.asarray(Wk, f), np.asarray(kvw, f))
    Wvc = np.einsum("jc,ciyx->jiyx", np.asarray(Wv, f), np.asarray(kvw, f))
    bqc = (np.asarray(Wq, f) @ np.asarray(qb, f) + np.asarray(bq, f)) * SCALE
    bkc = np.asarray(Wk, f) @ np.asarray(kvb, f) + np.asarray(bk, f)
    bvc = np.asarray(Wv, f) @ np.asarray(kvb, f) + np.asarray(bv, f)

    akv_p = np.zeros((DIM, 66, 66), f)
    akv_p[:, 1:65, 1:65] = akv
    x_p = np.zeros((DIM, 66, 66), f)
    x_p[:, 1:65, 1:65] = x

    per_pair = []
    for p in range(4):
        wkv = np.zeros((128, DIM, 3, 3), f)
        bkv = np.zeros((128,), f)
        wq_ = np.zeros((64, DIM, 3, 3), f)
        bq_ = np.zeros((64,), f)
        for hl in range(2):
            h = 2 * p + hl
            wkv[32 * hl:32 * hl + QK_DIM] = Wkc[QK_DIM * h:QK_DIM * (h + 1)]
            bkv[32 * hl:32 * hl + QK_DIM] = bkc[QK_DIM * h:QK_DIM * (h + 1)]
            wkv[64 + 32 * hl:64 + 32 * (hl + 1)] = Wvc[HEAD_DIM * h:HEAD_DIM * (h + 1)]
            bkv[64 + 32 * hl:64 + 32 * (hl + 1)] = bvc[HEAD_DIM * h:HEAD_DIM * (h + 1)]
            wq_[32 * hl:32 * hl + QK_DIM] = Wqc[QK_DIM * h:QK_DIM * (h + 1)]
            bq_[32 * hl:32 * hl + QK_DIM] = bqc[QK_DIM * h:QK_DIM * (h + 1)]
        per_pair.append((
            np.ascontiguousarray(wkv.transpose(1, 2, 3, 0).reshape(DIM, 9, 128)),
            bkv.reshape(1, 128),
            np.ascontiguousarray(wq_.transpose(1, 2, 3, 0).reshape(DIM, 9, 64)),
            bq_.reshape(1, 64),
        ))

    identsh = np.zeros((32, 4, 128), ml_dtypes.bfloat16)
    for jm in range(4):
        for dd in range(32):
            identsh[dd, jm, 32 * jm + dd] = 1.0
    wpt = np.ascontiguousarray(np.asarray(Wp, f).T).astype(ml_dtypes.bfloat16)
    bp_a = np.ascontiguousarray(np.asarray(bp, f).reshape(2, 128).T)

    in_maps = []
    for c in range(N_CORES):
        p, s = c // 2, c % 2
        wkv_h, bkv_h, wq_h, bq_h = per_pair[p]
        in_maps.append({
            "akv": akv_p.reshape(DIM, -1),
            "xs": np.ascontiguousarray(x_p[:, 32 * s:32 * s + 34, :]).reshape(DIM, -1),
            "wkv": wkv_h, "bkv": bkv_h, "wq": wq_h, "bq": bq_h,
            "ones512": np.ones((1, 512), f), "ones32": np.ones((1, 32), f),
            "ident64": np.eye(64, dtype=f), "identsh": identsh,
            "wpt": wpt, "bp": bp_a,
        })
    return in_maps


def gather(results):
    full = np.empty((DIM, N), np.float32)
    for c in range(N_CORES):
        p, s = c // 2, c % 2
        dev = results[c]["out"]
        for hl in range(2):
            h = 2 * p + hl
            full[:, 512 * h + 256 * s: 512 * h + 256 * s + 256] = \
                dev[:, 256 * hl:256 * hl + 256]
    return full.reshape(1, DIM, H, W)


def kernel(x, attn_kv, qw, qb, kvw, kvb, Wq, bq, Wk, bk, Wv, bv, Wp, bp):
    nc = build_nc()
    in_maps = host_prep(x, attn_kv, qw, qb, kvw, kvb, Wq, bq, Wk, bk, Wv, bv,
                        Wp, bp)
    res = bass_utils.run_bass_kernel_spmd(nc, in_maps,
                                          core_ids=list(range(N_CORES)),
                                          trace=False)
    return gather(res.results).astype(np.float32)

